# revision 1
# baseline (speedup 1.0000x reference)
"""CascadePredictor Trainium2 kernel: 2-layer GCN encode + collapsed MHA edge decode.

Distribution: 8-core SPMD, node-partitioned aggregation (load-balanced permuted
blocks), AllGather between layers, edge-parallel decode.

Algorithm (validated vs reference, numpy prototype):
  dinv[n] = 1/sqrt(indeg+1) (0 for pad nodes)
  hxd = (x @ W1 + b1) * dinv                       (bf16 table)
  h   = relu(dinv * (sum_{e: dst=d} hxd[src] + hxd[d]))
  hw2d= (h @ W2 + b2) * dinv                       (bf16 table, AllGather)
  z   = dinv * (sum hw2d[src] + hw2d[d])
  Tq  = [z@WqT*s | l0 | s0],  Tk = [z@WkT | s1]    (bf16 tables, AllGather)
  out = sigmoid(sum_h s0 + sigmoid(l1-l0)*(s1-s0) + bsum)   l1 = Q'[sp].K[dp]
"""
import sys
import numpy as np

for p in ("/opt/trn_rl_repo",):
    if p not in sys.path:
        sys.path.insert(0, p)

import ml_dtypes
import concourse.bass as bass
import concourse.bacc as bacc
import concourse.tile as tile
import concourse.mybir as mybir

bf16 = ml_dtypes.bfloat16
F32 = mybir.dt.float32
BF = mybir.dt.bfloat16
I32 = mybir.dt.int32

NCORES = 8
P = 128
HIDDEN = 256
NH, HD = 4, 64


# ----------------------------------------------------------------------------
# host-side preprocessing
# ----------------------------------------------------------------------------
def build_host_data(x, edge_index, edge_index_pred,
                    W1, b1, W2, b2, in_proj_w, in_proj_b, out_proj_w, out_proj_b):
    N = x.shape[0]
    src = np.asarray(edge_index[0], np.int64)
    dst = np.asarray(edge_index[1], np.int64)
    sp = np.asarray(edge_index_pred[0], np.int64)
    dp = np.asarray(edge_index_pred[1], np.int64)
    E = src.shape[0]
    EP = sp.shape[0]

    NBLK = -(-N // P)                      # blocks over real nodes
    NBLK = -(-NBLK // NCORES) * NCORES     # multiple of NCORES
    NPAD = NBLK * P
    NBC = NBLK // NCORES                   # blocks per core

    deg = np.bincount(dst, minlength=N).astype(np.float64) + 1.0
    dinv = np.zeros(NPAD, np.float32)
    dinv[:N] = (1.0 / np.sqrt(deg)).astype(np.float32)

    # --- load-balanced permutation: snake-assign nodes (sorted by indeg desc)
    indeg = (deg - 1.0).astype(np.int64)
    order = np.argsort(-indeg, kind="stable")
    snake = np.empty(N, np.int64)          # block id per sorted position
    pos = np.arange(N)
    rnd, off = pos // NBLK, pos % NBLK
    fwd = (rnd % 2) == 0
    snake[fwd] = off[fwd]
    snake[~fwd] = NBLK - 1 - off[~fwd]
    blk_of = np.empty(NPAD, np.int64)      # node -> block
    blk_of[order] = snake[:N]
    # pad nodes fill remaining slots
    slot_of = np.empty(NPAD, np.int64)
    # count real nodes per block, assign slots in order of appearance
    perm_sorted = np.argsort(blk_of[:N] * (NPAD + 1) + np.arange(N), kind="stable")
    # simpler: for each block, members = real nodes in it (<=P), then pads
    counts = np.bincount(blk_of[:N], minlength=NBLK)
    assert counts.max() <= P
    # stable order of real nodes by block
    o2 = np.argsort(blk_of[:N], kind="stable")
    within = np.arange(N) - np.repeat(np.concatenate([[0], np.cumsum(counts)[:-1]]), counts)
    slot_of[o2] = within
    # pads: fill blocks with free slots
    free_blocks = np.repeat(np.arange(NBLK), P - counts)
    pad_ids = np.arange(N, NPAD)
    blk_of[pad_ids] = free_blocks[: NPAD - N]
    pad_within = []
    fc = counts.copy()
    for b in free_blocks[: NPAD - N]:
        pad_within.append(fc[b])
        fc[b] += 1
    slot_of[pad_ids] = np.array(pad_within, np.int64) if len(pad_within) else np.zeros(0, np.int64)
    perm = blk_of * P + slot_of            # node -> permuted row
    assert np.array_equal(np.sort(perm), np.arange(NPAD))

    dinv_perm = np.zeros(NPAD, np.float32)
    dinv_perm[perm] = dinv                 # dinv for permuted rows (pads are 0)

    # --- edge grids: per block, edges grouped, padded; + self tile last
    pdst = perm[dst]
    psrc = perm[src]
    eblk = pdst // P
    eloc = pdst % P
    ecnt = np.bincount(eblk, minlength=NBLK)
    TE = int(-(-ecnt.max() // P))          # edge tiles per block
    T = TE                                 # self-loop handled via shard DMA
    eord = np.argsort(eblk, kind="stable")
    starts = np.concatenate([[0], np.cumsum(ecnt)[:-1]])
    epos = np.arange(E) - np.repeat(starts, ecnt)
    gsrc = np.zeros((NBLK, P, T), np.int32)
    dstloc = np.full((NBLK, P, T), -1.0, np.float32)
    b_, p_, t_ = eblk[eord], (epos % P), (epos // P)
    gsrc[b_, p_, t_] = psrc[eord].astype(np.int32)
    dstloc[b_, p_, t_] = eloc[eord].astype(np.float32)

    # per-core resident layouts [P, NBC*T]
    g4 = gsrc.reshape(NCORES, NBC, P, T)
    d4 = dstloc.reshape(NCORES, NBC, P, T)
    gsrc_core = [np.ascontiguousarray(g4[c].transpose(1, 0, 2).reshape(P, NBC * T)) for c in range(NCORES)]
    dstloc_core = [np.ascontiguousarray(d4[c].transpose(1, 0, 2).reshape(P, NBC * T)).astype(bf16) for c in range(NCORES)]

    # --- decode edge split: sp-sorted tiles whose sp-panels fit a sliding
    # window [phi(t), phi(t)+KW), so the Q side streams from sequential panels.
    KW = 3
    EPC_raw = -(-EP // NCORES)
    core_psp, core_pdp, core_orig = [], [], []
    for c in range(NCORES):
        lo, hi = c * EPC_raw, min((c + 1) * EPC_raw, EP)
        ps_ = perm[sp[lo:hi]]
        od = np.argsort(ps_, kind="stable")
        core_psp.append(ps_[od])
        core_pdp.append(perm[dp[lo:hi]][od])
        core_orig.append(np.arange(lo, hi)[od])

    def try_pack(pj, NDT2):
        nslots = NDT2 * P
        slot_edge = np.full(nslots, -1, np.int64)
        t, slot = 0, 0
        for i, j in enumerate(pj):
            while True:
                if t >= NDT2:
                    return None
                phi = (t * NBLK) // NDT2
                if j < phi:
                    return None
                if j >= phi + KW:
                    t += 1
                    slot = 0
                    continue
                break
            slot_edge[t * P + slot] = i
            slot += 1
            if slot == P:
                t += 1
                slot = 0
        return slot_edge

    base = -(-EPC_raw // P)
    base = -(-base // 4) * 4
    NDT = None
    for cand in range(base, base + 64, 4):
        packs = [try_pack(core_psp[c] // P, cand) for c in range(NCORES)]
        if all(pk is not None for pk in packs):
            NDT = cand
            break
    assert NDT is not None, "decode window packing failed"
    EPC = NDT * P
    sploc_core, dpi, invmap = [], [], []
    for c in range(NCORES):
        pk = packs[c]
        valid = pk >= 0
        psp_s = np.where(valid, core_psp[c][np.maximum(pk, 0)], -1)
        dp_s = np.where(valid, core_pdp[c][np.maximum(pk, 0)], 0)
        inv = np.where(valid, core_orig[c][np.maximum(pk, 0)], -1)
        # sploc[t, slot, k]: row within panel phi(t)+k, else -1
        sl = np.full((NDT, P, KW), -1.0, np.float32)
        tt = np.arange(NDT)
        phis = (tt * NBLK) // NDT
        pj = psp_s.reshape(NDT, P) // P
        pr = psp_s.reshape(NDT, P) % P
        for k in range(KW):
            hit = (pj == (phis[:, None] + k)) & (psp_s.reshape(NDT, P) >= 0)
            sl[:, :, k] = np.where(hit, pr, -1).astype(np.float32)
        sploc_core.append(np.ascontiguousarray(
            sl.transpose(1, 0, 2).reshape(P, NDT * KW)).astype(bf16))
        if c == 0:
            active = (sl >= 0).any(axis=1)
        else:
            active |= (sl >= 0).any(axis=1)
        dpi.append(np.ascontiguousarray(
            dp_s.reshape(NDT, P).T).astype(np.int32))
        invmap.append(inv)

    # --- dense weights / tables
    xp = np.zeros((NPAD, x.shape[1]), np.float32)
    xp[perm[:N]] = np.asarray(x, np.float32)[:N]  # permuted rows
    xT = np.ascontiguousarray(xp.T).astype(bf16)  # [IN_CH, NPAD]

    dinv_cols = np.ascontiguousarray(dinv_perm.reshape(NBLK, P).T)  # [P, NBLK] f32

    H = HIDDEN
    Wq = in_proj_w[0:H]; Wk = in_proj_w[H:2 * H]; Wv = in_proj_w[2 * H:3 * H]
    bq = in_proj_b[0:H]; bk = in_proj_b[H:2 * H]; bv = in_proj_b[2 * H:3 * H]
    c_vec = out_proj_w.sum(axis=0)
    bsum = float(out_proj_b.sum())
    scale = 1.0 / np.sqrt(HD)
    u2 = np.stack([(Wv[h * HD:(h + 1) * HD, :] * c_vec[h * HD:(h + 1) * HD, None]).sum(0)
                   for h in range(NH)], axis=1)      # [256, 4]
    beta = np.stack([(bv[h * HD:(h + 1) * HD] * c_vec[h * HD:(h + 1) * HD]).sum()
                     for h in range(NH)])            # [4]

    KIN = x.shape[1]
    assert KIN == P, "stage A assumes IN_CH == 128"
    meta = dict(NPAD=NPAD, NBLK=NBLK, NBC=NBC, T=T, TE=TE, NDT=NDT, EPC=EPC,
                EPC_raw=EPC_raw, EP=EP, bsum=bsum, KW=KW, invmap=invmap,
                active=tuple(map(tuple, active)))

    common = {
        "dinv_cols": dinv_cols.astype(np.float32),
        "w1": np.asarray(W1, np.float32).astype(bf16),                      # [128,256]
        "w2c": np.asarray(W2, np.float32).reshape(2, P, H).astype(bf16),    # chunks of rows
        "wqc": (np.asarray(Wq, np.float32).T * scale).reshape(2, P, H).astype(bf16),
        "wkc": np.asarray(Wk, np.float32).T.reshape(2, P, H).astype(bf16),
        "uc": u2.reshape(2, P, NH).astype(bf16),
        "b1r": np.asarray(b1, np.float32).reshape(1, H).astype(bf16),
        "b2r": np.asarray(b2, np.float32).reshape(1, H).astype(bf16),
        "bqr": (np.asarray(bq, np.float32) * scale).reshape(1, H).astype(bf16),
        "bkr": np.asarray(bk, np.float32).reshape(1, H).astype(bf16),
        "betar": beta.reshape(1, NH).astype(np.float32),
        "iota_row": np.tile(np.arange(P, dtype=np.float32).astype(bf16)[None, :], (P, 1)),
        "ident_bf": np.eye(P, dtype=np.float32).astype(bf16),
        "ident_f32": np.eye(P, dtype=np.float32),
    }
    in_maps = []
    for c in range(NCORES):
        m = dict(common)
        m["xT"] = np.ascontiguousarray(xT[:, c * NBC * P:(c + 1) * NBC * P])
        m["gsrc"] = gsrc_core[c]
        m["dstloc"] = dstloc_core[c]
        m["dinv_own"] = np.ascontiguousarray(dinv_cols[:, c * NBC:(c + 1) * NBC]).astype(np.float32)
        m["sploc"] = sploc_core[c]
        m["dpidx"] = dpi[c]
        in_maps.append(m)
    return in_maps, meta


# ----------------------------------------------------------------------------
# program builder
# ----------------------------------------------------------------------------
def build_program(meta):
    NPAD, NBLK, NBC, T, TE, NDT, KW = (meta[k] for k in
                                   ("NPAD", "NBLK", "NBC", "T", "TE", "NDT", "KW"))
    H = HIDDEN
    TW = 264  # packed table width

    nc = bacc.Bacc("TRN2", target_bir_lowering=False, debug=False,
                   num_devices=NCORES)

    def din(name, shape, dt):
        return nc.dram_tensor(name, shape, dt, kind="ExternalInput")

    xT = din("xT", [P, NBC * P], BF)
    dinv_cols = din("dinv_cols", [P, NBLK], F32)
    dinv_own = din("dinv_own", [P, NBC], F32)
    w1 = din("w1", [P, H], BF)
    w2c = din("w2c", [2, P, H], BF)
    wqc = din("wqc", [2, P, H], BF)
    wkc = din("wkc", [2, P, H], BF)
    uc = din("uc", [2, P, NH], BF)
    b1r = din("b1r", [1, H], BF)
    b2r = din("b2r", [1, H], BF)
    bqr = din("bqr", [1, H], BF)
    bkr = din("bkr", [1, H], BF)
    betar = din("betar", [1, NH], F32)
    iota_in = din("iota_row", [P, P], BF)
    identb_in = din("ident_bf", [P, P], BF)
    identf_in = din("ident_f32", [P, P], F32)
    gsrc_in = din("gsrc", [P, NBC * T], I32)
    dstloc_in = din("dstloc", [P, NBC * T], BF)
    sploc_in = din("sploc", [P, NDT * KW], BF)
    dpidx_in = din("dpidx", [P, NDT], I32)

    out_t = nc.dram_tensor("out", [NDT * P], F32, kind="ExternalOutput")

    hxd_shard = nc.dram_tensor("hxd_shard", [NBC * P, H], BF, kind="Internal")
    hxd = nc.dram_tensor("hxd", [NPAD, H], BF, kind="Internal", addr_space="Shared")
    hw2d_shard = nc.dram_tensor("hw2d_shard", [NBC * P, H], BF, kind="Internal")
    hw2d_full = nc.dram_tensor("hw2d_full", [NPAD, H], BF, kind="Internal", addr_space="Shared")
    tqk_shard = nc.dram_tensor("tqk_shard", [NBC * P, 2 * TW], BF, kind="Internal")
    tqk_full = nc.dram_tensor("tqk_full", [NPAD, 2 * TW], BF, kind="Internal", addr_space="Shared")

    AG = mybir.AluOpType
    with tile.TileContext(nc) as tc:
        with tc.tile_pool(name="sb", bufs=1) as res, \
             tc.tile_pool(name="wk", bufs=3) as wk, \
             tc.tile_pool(name="gp", bufs=12) as gp, \
             tc.tile_pool(name="ps", bufs=4, space="PSUM") as psp, \
             tc.tile_pool(name="pt", bufs=2, space="PSUM") as ptp:

            # ---------------- residents
            def load(name, src, shape, dt):
                t = res.tile(shape, dt, tag=name)
                nc.sync.dma_start(t[:], src[:])
                return t
            w1_t = load("w1", w1, [P, H], BF)

            def load2(name, src, width, dt):
                # [2, P, width] dram chunks -> [P, 2*width] sbuf
                t = res.tile([P, 2 * width], dt, tag=name)
                for k in range(2):
                    nc.sync.dma_start(t[:, k * width:(k + 1) * width], src[k])
                return t
            w2_t = load2("w2c", w2c, H, BF)
            wq_t = load2("wqc", wqc, H, BF)
            wk_t = load2("wkc", wkc, H, BF)
            uc_t = load2("uc", uc, NH, BF)
            iota_t = load("iota", iota_in, [P, P], BF)
            idb_t = load("idb", identb_in, [P, P], BF)
            idf_t = load("idf", identf_in, [P, P], F32)
            dinvc_t = load("dinvc", dinv_cols, [P, NBLK], F32)
            dinvo_t = load("dinvo", dinv_own, [P, NBC], F32)
            gsrc_t = load("gsrc", gsrc_in, [P, NBC * T], I32)
            dstloc_t = load("dstloc", dstloc_in, [P, NBC * T], BF)
            sploc_t = load("sploc", sploc_in, [P, NDT * KW], BF)
            dpidx_t = load("dpidx", dpidx_in, [P, NDT], I32)
            # biases broadcast to 128 partitions via DMA
            def loadb(name, src):
                t = res.tile([P, H], BF, tag=name)
                nc.sync.dma_start(t[:], src[:].to_broadcast((P, H)))
                return t
            b1_t = loadb("b1", b1r)
            b2_t = loadb("b2", b2r)
            bq_t = loadb("bq", bqr)
            bk_t = loadb("bk", bkr)
            beta_b = res.tile([P, NH], F32, tag="betab")
            nc.sync.dma_start(beta_b[:], betar[:].to_broadcast((P, NH)))

            colbuf = res.tile([P, NDT], F32, tag="colbuf")
            bsum_t = res.tile([P, 1], F32, tag="bsum")
            nc.vector.memset(bsum_t[:], float(meta["bsum"]))

            # ---------------- stage A: hxd = (x @ W1 + b1) * dinv  (own shard only)
            QUAD = 4
            for i0 in range(0, NBC, QUAD):
                nq = min(QUAD, NBC - i0)
                xt = wk.tile([P, QUAD * P], BF, tag="xt")
                nc.sync.dma_start(xt[:, :nq * P], xT[:, i0 * P:(i0 + nq) * P])
                for j in range(nq):
                    i = i0 + j
                    ps = psp.tile([P, H], F32, tag="p256", space="PSUM")
                    nc.tensor.matmul(ps[:], lhsT=xt[:, j * P:(j + 1) * P], rhs=w1_t[:],
                                     start=True, stop=True)
                    tmp = wk.tile([P, H], F32, tag="tmpA")
                    nc.vector.tensor_tensor(out=tmp[:], in0=ps[:], in1=b1_t[:], op=AG.add)
                    hx = wk.tile([P, H], BF, tag="hx")
                    nc.scalar.activation(hx[:], tmp[:], mybir.ActivationFunctionType.Copy,
                                         scale=dinvo_t[:, i:i + 1])
                    nc.sync.dma_start(hxd_shard[i * P:(i + 1) * P, :], hx[:])
            nc.gpsimd.collective_compute(
                "AllGather", AG.bypass, replica_groups=[list(range(NCORES))],
                ins=[hxd_shard[:]], outs=[hxd[:]])

            # ---------------- aggregation layer template
            def agg_layer(table, shard, b, finalize):
                agg = psp.tile([P, H], F32, tag="p256", space="PSUM")
                for t in range(T):
                    col = b * T + t
                    g = gp.tile([P, H], BF, tag="g")
                    nc.gpsimd.indirect_dma_start(
                        out=g[:], out_offset=None, in_=table[:],
                        in_offset=bass.IndirectOffsetOnAxis(ap=gsrc_t[:, col:col + 1], axis=0))
                    st = gp.tile([P, P], BF, tag="st")
                    nc.vector.tensor_tensor(
                        out=st[:], in0=iota_t[:],
                        in1=dstloc_t[:, col:col + 1].to_broadcast((P, P)), op=AG.is_equal)
                    nc.tensor.matmul(agg[:], lhsT=st[:], rhs=g[:],
                                     start=(t == 0), stop=(t == T - 1))
                selfb = wk.tile([P, H], BF, tag="selfb")
                nc.sync.dma_start(selfb[:], shard[b * P:(b + 1) * P, :])
                asum = wk.tile([P, H], F32, tag="asum")
                nc.vector.tensor_tensor(out=asum[:], in0=agg[:], in1=selfb[:], op=AG.add)
                finalize(asum)

            def transposed_chunks(src_bf, tag):
                outs = []
                for k in range(2):
                    pt = ptp.tile([P, P], BF, tag="pT", space="PSUM")
                    nc.tensor.transpose(pt[:], src_bf[:, k * P:(k + 1) * P], idb_t[:])
                    sb = wk.tile([P, P], BF, tag=f"{tag}{k}")
                    nc.vector.tensor_copy(out=sb[:], in_=pt[:])
                    outs.append(sb)
                return outs

            # ---------------- layer 1 + transform
            for b in range(NBC):
                def fin1(agg, b=b):
                    h1 = wk.tile([P, H], BF, tag="h1")
                    nc.scalar.activation(h1[:], agg[:], mybir.ActivationFunctionType.Relu,
                                         scale=dinvo_t[:, b:b + 1])
                    hts = transposed_chunks(h1, "h1T")
                    ps2 = psp.tile([P, H], F32, tag="p256", space="PSUM")
                    for k in range(2):
                        nc.tensor.matmul(ps2[:], lhsT=hts[k][:], rhs=w2_t[:, k * H:(k + 1) * H],
                                         start=(k == 0), stop=(k == 1))
                    t2 = wk.tile([P, H], F32, tag="t2")
                    nc.vector.tensor_tensor(out=t2[:], in0=ps2[:], in1=b2_t[:], op=AG.add)
                    hwb = wk.tile([P, H], BF, tag="hwb")
                    nc.scalar.activation(hwb[:], t2[:], mybir.ActivationFunctionType.Copy,
                                         scale=dinvo_t[:, b:b + 1])
                    nc.sync.dma_start(hw2d_shard[b * P:(b + 1) * P, :], hwb[:])
                agg_layer(hxd, hxd_shard, b, fin1)

            nc.gpsimd.collective_compute(
                "AllGather", AG.bypass, replica_groups=[list(range(NCORES))],
                ins=[hw2d_shard[:]], outs=[hw2d_full[:]])

            # ---------------- layer 2 + decode tables
            for b in range(NBC):
                def fin2(agg, b=b):
                    zb = wk.tile([P, H], BF, tag="zb")
                    nc.scalar.activation(zb[:], agg[:], mybir.ActivationFunctionType.Copy,
                                         scale=dinvo_t[:, b:b + 1])
                    zts = transposed_chunks(zb, "zT")
                    tqkb = wk.tile([P, 2 * TW], BF, tag="tqkb")
                    tqb = tqkb[:, 0:TW]
                    tkb = tqkb[:, TW:2 * TW]
                    # Q' = z@WqT*s + bq'
                    psq = psp.tile([P, H], F32, tag="p256", space="PSUM")
                    for k in range(2):
                        nc.tensor.matmul(psq[:], lhsT=zts[k][:], rhs=wq_t[:, k * H:(k + 1) * H],
                                         start=(k == 0), stop=(k == 1))
                    nc.vector.tensor_tensor(out=tqb[:, 0:H], in0=psq[:], in1=bq_t[:], op=AG.add)
                    # K = z@WkT + bk
                    psk = psp.tile([P, H], F32, tag="p256", space="PSUM")
                    for k in range(2):
                        nc.tensor.matmul(psk[:], lhsT=zts[k][:], rhs=wk_t[:, k * H:(k + 1) * H],
                                         start=(k == 0), stop=(k == 1))
                    nc.vector.tensor_tensor(out=tkb[:, 0:H], in0=psk[:], in1=bk_t[:], op=AG.add)
                    # l0 per head
                    qk = wk.tile([P, H], F32, tag="qk")
                    nc.vector.tensor_tensor(out=qk[:], in0=tqb[:, 0:H], in1=tkb[:, 0:H], op=AG.mult)
                    l0 = wk.tile([P, NH], F32, tag="l0")
                    nc.vector.tensor_reduce(out=l0[:], in_=qk[:].rearrange("p (h d) -> p h d", h=NH),
                                            axis=mybir.AxisListType.X, op=AG.add)
                    nc.vector.tensor_copy(out=tqb[:, H:H + NH], in_=l0[:])
                    # S per head
                    pss = ptp.tile([P, NH], F32, tag="pS", space="PSUM")
                    for k in range(2):
                        nc.tensor.matmul(pss[:], lhsT=zts[k][:], rhs=uc_t[:, k * NH:(k + 1) * NH],
                                         start=(k == 0), stop=(k == 1))
                    sf = wk.tile([P, NH], F32, tag="sf")
                    nc.vector.tensor_tensor(out=sf[:], in0=pss[:], in1=beta_b[:], op=AG.add)
                    nc.vector.tensor_copy(out=tqb[:, H + NH:H + 2 * NH], in_=sf[:])
                    nc.vector.tensor_copy(out=tkb[:, H:H + NH], in_=sf[:])
                    nc.vector.memset(tkb[:, H + NH:TW], 0)
                    nc.sync.dma_start(tqk_shard[b * P:(b + 1) * P, :], tqkb[:])
                agg_layer(hw2d_full, hw2d_shard, b, fin2)

            nc.gpsimd.collective_compute(
                "AllGather", AG.bypass, replica_groups=[list(range(NCORES))],
                ins=[tqk_shard[:]], outs=[tqk_full[:]])

            # ---------------- decode (Q side streamed from panels, K side gathered)
            DG = 4  # tiles per vector batch
            assert NDT % DG == 0
            W = KW + 2
            panelbuf = res.tile([P, W * TW], BF, tag="panelbuf")
            next_p = 0
            for g0 in range(0, NDT, DG):
                gq = wk.tile([P, DG, TW], BF, tag="gq")
                gk = gp.tile([P, DG, TW], BF, tag="gk")
                for j in range(DG):
                    t = g0 + j
                    phi_t = (t * NBLK) // NDT
                    while next_p < min(phi_t + KW, NBLK):
                        nc.sync.dma_start(
                            panelbuf[:, (next_p % W) * TW:(next_p % W + 1) * TW],
                            tqk_full[next_p * P:(next_p + 1) * P, 0:TW])
                        next_p += 1
                    nc.gpsimd.indirect_dma_start(
                        out=gk[:, j, :], out_offset=None, in_=tqk_full[:],
                        in_offset=bass.IndirectOffsetOnAxis(ap=dpidx_t[:, t:t + 1], axis=0),
                        element_offset=TW)
                    psq = psp.tile([P, TW], F32, tag="p256", space="PSUM")
                    ks = [k for k in range(KW)
                          if phi_t + k < NBLK and meta["active"][t][k]]
                    if not ks:
                        ks = [0]
                    for ki, k in enumerate(ks):
                        p = phi_t + k
                        rt = gp.tile([P, P], BF, tag="rt")
                        nc.vector.tensor_tensor(
                            out=rt[:], in0=iota_t[:],
                            in1=sploc_t[:, t * KW + k:t * KW + k + 1].to_broadcast((P, P)),
                            op=AG.is_equal)
                        prt = ptp.tile([P, P], BF, tag="pT", space="PSUM")
                        nc.tensor.transpose(prt[:], rt[:], idb_t[:])
                        Rb = gp.tile([P, P], BF, tag="Rb")
                        nc.vector.tensor_copy(out=Rb[:], in_=prt[:])
                        nc.tensor.matmul(psq[:], lhsT=Rb[:],
                                         rhs=panelbuf[:, (p % W) * TW:(p % W) * TW + TW],
                                         start=(ki == 0), stop=(ki == len(ks) - 1))
                    nc.vector.tensor_copy(out=gq[:, j, :], in_=psq[:])
                prod = wk.tile([P, DG, H], F32, tag="prod")
                nc.vector.tensor_tensor(out=prod[:], in0=gq[:, :, 0:H], in1=gk[:, :, 0:H], op=AG.mult)
                l1 = wk.tile([P, DG * NH], F32, tag="l1")
                nc.vector.tensor_reduce(out=l1[:], in_=prod[:].rearrange("p g (h d) -> p (g h) d", h=NH),
                                        axis=mybir.AxisListType.X, op=AG.add)
                dlt = wk.tile([P, DG * NH], F32, tag="dlt")
                nc.vector.tensor_tensor(out=dlt[:].rearrange("p (g h) -> p g h", h=NH),
                                        in0=l1[:].rearrange("p (g h) -> p g h", h=NH),
                                        in1=gq[:, :, H:H + NH], op=AG.subtract)
                a1 = wk.tile([P, DG * NH], F32, tag="a1")
                nc.scalar.activation(a1[:], dlt[:], mybir.ActivationFunctionType.Sigmoid)
                ds = wk.tile([P, DG * NH], F32, tag="ds")
                nc.vector.tensor_tensor(out=ds[:].rearrange("p (g h) -> p g h", h=NH),
                                        in0=gk[:, :, H:H + NH],
                                        in1=gq[:, :, H + NH:H + 2 * NH],
                                        op=AG.subtract)
                pr = wk.tile([P, DG * NH], F32, tag="pr")
                nc.vector.tensor_tensor(out=pr[:], in0=a1[:], in1=ds[:], op=AG.mult)
                prs = wk.tile([P, DG], F32, tag="prs")
                nc.vector.tensor_reduce(out=prs[:], in_=pr[:].rearrange("p (g h) -> p g h", h=NH),
                                        axis=mybir.AxisListType.X, op=AG.add)
                s0s = wk.tile([P, DG], F32, tag="s0s")
                nc.vector.tensor_reduce(out=s0s[:], in_=gq[:, :, H + NH:H + 2 * NH],
                                        axis=mybir.AxisListType.X, op=AG.add)
                rr = wk.tile([P, DG], F32, tag="rr")
                nc.vector.tensor_tensor(out=rr[:], in0=prs[:], in1=s0s[:], op=AG.add)
                nc.scalar.activation(colbuf[:, g0:g0 + DG], rr[:],
                                     mybir.ActivationFunctionType.Sigmoid, bias=bsum_t[:])

            # transpose colbuf -> out
            for c0 in range(0, NDT, P):
                w = min(P, NDT - c0)
                po = ptp.tile([P, P], F32, tag="pT", space="PSUM")
                nc.tensor.transpose(po[:w, :], colbuf[:, c0:c0 + w], idf_t[:])
                ob = wk.tile([P, P], F32, tag="ob")
                nc.vector.tensor_copy(out=ob[:w, :], in_=po[:w, :])
                nc.sync.dma_start(
                    out_t[c0 * P:(c0 + w) * P].rearrange("(a b) -> a b", b=P), ob[:w, :])
    nc.compile()
    return nc


# ----------------------------------------------------------------------------
_CACHE = {}


TRACE = False
LAST_EXEC_NS = None


def kernel(**inputs):
    import concourse.bass_utils as bass_utils
    global LAST_EXEC_NS
    in_maps, meta = build_host_data(**inputs)
    key = (meta["NPAD"], meta["NBLK"], meta["T"], meta["NDT"], hash(meta["active"]))
    if key not in _CACHE:
        _CACHE[key] = build_program(meta)
    nc = _CACHE[key]
    trace = bool(TRACE)
    if trace:
        try:
            from trn_agent_boot.trn_boot import _ntff_profile_via_ctypes
            import antenv.axon_hooks as ah
            if ah.get_axon_ntff_profile_hook() is None:
                ah.set_axon_ntff_profile_hook(
                    _ntff_profile_via_ctypes("/opt/axon/libaxon_pjrt.so"))
        except Exception:
            trace = False
    res = bass_utils.run_bass_kernel_spmd(nc, in_maps, core_ids=list(range(NCORES)),
                                          trace=trace)
    LAST_EXEC_NS = res.exec_time_ns
    EP = meta["EP"]
    out = np.zeros(EP, np.float32)
    for c in range(NCORES):
        inv = meta["invmap"][c]
        m = inv >= 0
        out[inv[m]] = res.results[c]["out"][m]
    return out



# revision 2
# speedup vs baseline: 2.5356x; 2.5356x over previous
"""CascadePredictor Trainium2 kernel v2: 2-layer GCN encode + collapsed MHA
edge decode.

Distribution: 8-core SPMD, node-partitioned aggregation (load-balanced
permuted blocks), edge-parallel decode sharded by sp-owner core.

v2 structure (vs v1): all gathers batched via gpsimd.dma_gather (int16 idx,
half-split tables to fit int16); layer-1 aggregates u = dinv*x BEFORE the W1
transform (halves gather bytes, removes stage A and the first AllGather);
transposed PSUM aggregation avoids per-block PE transposes; decode gathers
both edge endpoints (no panel/selector machinery) with the Q-side table kept
core-local (only the K-side table is AllGathered).

Math (validated vs reference in numpy):
  u[n]     = dinv[n] * x[n]                          (host, bf16 table)
  aggT     = sum_{e: dst=d} u[src_e]   (incl. self loop, via selector matmul)
  h1raw    = relu(W1^T aggT + b1 (x) sigma)          sigma[d] = sum dinv[src]
  hw2d[d]  = dinv^2[d] * (h1raw^T W2)[d] + dinv[d]*b2     (bf16 table, AG)
  agg2T    = sum hw2d[src_e]          z^T = dinv[d] * agg2T
  tq[d]    = [z Wq^T s + bq s | l0 | s0],  tk[d] = [z Wk^T + bk | s1]
  out      = sigmoid(sum_h s0 + sigmoid(l1-l0)*(s1-s0) + bsum),
             l1 = tq[sp] . tk[dp] per head
"""
import os
import sys
import numpy as np

for p in ("/opt/trn_rl_repo",):
    if p not in sys.path:
        sys.path.insert(0, p)

# bisect knob: 1 = L1 only, 2 = +AG+L2, 3 = full (default)
STAGE = int(os.environ.get("V2_STAGE", "3"))

import ml_dtypes
import concourse.bass as bass
import concourse.bacc as bacc
import concourse.tile as tile
import concourse.mybir as mybir

bf16 = ml_dtypes.bfloat16
F32 = mybir.dt.float32
BF = mybir.dt.bfloat16
I16 = mybir.dt.int16

NCORES = 8
P = 128
HIDDEN = 256
NH, HD = 4, 64
DG = 8          # decode tiles per group
TW = 264        # written table row width
TROW = 384      # padded table row stride (768B, 256B-multiple for dma_gather)


def _pack_idx16(vals):
    n = len(vals)
    assert n % 16 == 0
    a = np.asarray(vals).reshape(n // 16, 16).T.astype(np.int16)
    return np.tile(a, (8, 1))          # [128, n/16]


def _padm(v, m=P):
    k = (-len(v)) % m
    if k:
        v = np.concatenate([v, np.zeros(k, v.dtype)])
    return v


# ----------------------------------------------------------------------------
# host-side preprocessing
# ----------------------------------------------------------------------------
def build_host_data(x, edge_index, edge_index_pred,
                    W1, b1, W2, b2, in_proj_w, in_proj_b, out_proj_w, out_proj_b):
    N = x.shape[0]
    assert x.shape[1] == P
    src = np.asarray(edge_index[0], np.int64)
    dst = np.asarray(edge_index[1], np.int64)
    sp = np.asarray(edge_index_pred[0], np.int64)
    dp = np.asarray(edge_index_pred[1], np.int64)
    EP = sp.shape[0]

    NBLK = -(-N // P)
    NBLK = -(-NBLK // NCORES) * NCORES
    NPAD = NBLK * P
    NBC = NBLK // NCORES
    HALF = NPAD // 2
    assert HALF < 32768

    deg = np.bincount(dst, minlength=N).astype(np.float64) + 1.0
    dinv = np.zeros(NPAD, np.float32)
    dinv[:N] = (1.0 / np.sqrt(deg)).astype(np.float32)

    # --- load-balanced permutation: snake-assign nodes (sorted by indeg desc)
    indeg = (deg - 1.0).astype(np.int64)
    order = np.argsort(-indeg, kind="stable")
    snake = np.empty(N, np.int64)
    pos = np.arange(N)
    rnd, off = pos // NBLK, pos % NBLK
    fwd = (rnd % 2) == 0
    snake[fwd] = off[fwd]
    snake[~fwd] = NBLK - 1 - off[~fwd]
    blk_of = np.empty(NPAD, np.int64)
    blk_of[order] = snake[:N]
    slot_of = np.empty(NPAD, np.int64)
    counts = np.bincount(blk_of[:N], minlength=NBLK)
    assert counts.max() <= P
    o2 = np.argsort(blk_of[:N], kind="stable")
    within = np.arange(N) - np.repeat(
        np.concatenate([[0], np.cumsum(counts)[:-1]]), counts)
    slot_of[o2] = within
    free_blocks = np.repeat(np.arange(NBLK), P - counts)
    pad_ids = np.arange(N, NPAD)
    blk_of[pad_ids] = free_blocks[: NPAD - N]
    fc = counts.copy()
    pw = []
    for b in free_blocks[: NPAD - N]:
        pw.append(fc[b])
        fc[b] += 1
    slot_of[pad_ids] = np.array(pw, np.int64) if pw else np.zeros(0, np.int64)
    perm = blk_of * P + slot_of
    assert np.array_equal(np.sort(perm), np.arange(NPAD))

    dinv_perm = np.zeros(NPAD, np.float32)
    dinv_perm[perm] = dinv

    # --- extended edge list (graph + self loops for real nodes)
    psrc = np.concatenate([perm[src], perm[np.arange(N)]])
    pdst = np.concatenate([perm[dst], perm[np.arange(N)]])
    sigma_perm = np.bincount(
        pdst, weights=dinv_perm[psrc].astype(np.float64),
        minlength=NPAD).astype(np.float32)

    eblk = pdst // P
    eloc = pdst % P
    ishigh = (psrc >= HALF).astype(np.int64)
    cnt_lo = np.bincount(eblk[ishigh == 0], minlength=NBLK)
    cnt_hi = np.bincount(eblk[ishigh == 1], minlength=NBLK)
    TLb = -(-cnt_lo // P)
    THb = -(-cnt_hi // P)
    TL = TLb.reshape(NCORES, NBC).max(axis=0)      # uniform per block slot
    TH = THb.reshape(NCORES, NBC).max(axis=0)
    TT = int((TL + TH).sum())

    # batches of 2 block-slots; chunk/column layout per batch:
    # [lows j0, lows j1, highs j0, highs j1]
    batches = []
    jj = 0
    col = 0
    col_lo = np.zeros(NBC, np.int64)
    col_hi = np.zeros(NBC, np.int64)
    while jj < NBC:
        bl = list(range(jj, min(jj + 2, NBC)))
        base = col
        for j in bl:
            col_lo[j] = col
            col += int(TL[j])
        for j in bl:
            col_hi[j] = col
            col += int(TH[j])
        batches.append(dict(blocks=bl, base=base,
                            nlo=int(TL[bl[0]:bl[-1] + 1].sum()),
                            nhi=int(TH[bl[0]:bl[-1] + 1].sum())))
        jj += 2
    assert col == TT
    TBMAX = max(b["nlo"] + b["nhi"] for b in batches)
    TJMAX = int((TL + TH).max())

    # per-core agg tables
    eord = np.argsort(eblk * 2 + ishigh, kind="stable")
    gb_sorted = eblk[eord]
    gs_sorted = psrc[eord]
    gl_sorted = eloc[eord]
    gh_sorted = ishigh[eord]
    # start offset of each (block, half) group
    grp = gb_sorted * 2 + gh_sorted
    grp_counts = np.bincount(grp, minlength=2 * NBLK)
    grp_starts = np.concatenate([[0], np.cumsum(grp_counts)[:-1]])

    gvals_core = []
    dloc_core = []
    for c in range(NCORES):
        gvals = np.zeros(TT * P, np.int64)
        dloc = np.full((P, TT), -1.0, np.float32)
        for j in range(NBC):
            b = c * NBC + j
            for half, colbase, in ((0, col_lo[j]), (1, col_hi[j])):
                g = 2 * b + half
                s0, n = grp_starts[g], grp_counts[g]
                vals = gs_sorted[s0:s0 + n] - half * HALF
                locs = gl_sorted[s0:s0 + n]
                i = np.arange(n)
                gvals[colbase * P + i] = vals
                dloc[i % P, colbase + i // P] = locs
        gvals_core.append(_pack_idx16(gvals))          # [128, TT*8]
        dloc_core.append(np.ascontiguousarray(dloc).astype(bf16))

    # --- decode: shard by sp-owner core, split by dp half
    psp = perm[sp]
    pdp = perm[dp]
    core_of = psp // (NBC * P)
    mls, mhs = [], []
    for c in range(NCORES):
        m = np.where(core_of == c)[0]
        mls.append(m[pdp[m] < HALF])
        mhs.append(m[pdp[m] >= HALF])
    NDTL = max(-(-len(m) // P) for m in mls)
    NDTH = max(-(-len(m) // P) for m in mhs)
    NDTL = -(-NDTL // DG) * DG
    NDTH = -(-NDTH // DG) * DG
    NDT = NDTL + NDTH

    spidx_core, dpidx_core, invmap = [], [], []
    for c in range(NCORES):
        spv = np.zeros(NDT * P, np.int64)
        dpv = np.zeros(NDT * P, np.int64)
        inv = np.full(NDT * P, -1, np.int64)
        ml, mh = mls[c], mhs[c]
        spv[:len(ml)] = psp[ml] - c * NBC * P
        dpv[:len(ml)] = pdp[ml]
        inv[:len(ml)] = ml
        o = NDTL * P
        spv[o:o + len(mh)] = psp[mh] - c * NBC * P
        dpv[o:o + len(mh)] = pdp[mh] - HALF
        inv[o:o + len(mh)] = mh
        spidx_core.append(_pack_idx16(spv))
        dpidx_core.append(_pack_idx16(dpv))
        invmap.append(inv)

    # --- dense weights
    H = HIDDEN
    Wq = in_proj_w[0:H]; Wk = in_proj_w[H:2 * H]; Wv = in_proj_w[2 * H:3 * H]
    bq = in_proj_b[0:H]; bk = in_proj_b[H:2 * H]; bv = in_proj_b[2 * H:3 * H]
    c_vec = out_proj_w.sum(axis=0)
    bsum = float(out_proj_b.sum())
    scale = 1.0 / np.sqrt(HD)
    u2 = np.stack(
        [(Wv[h * HD:(h + 1) * HD, :] * c_vec[h * HD:(h + 1) * HD, None]).sum(0)
         for h in range(NH)], axis=1)                  # [256, 4]
    beta = np.stack([(bv[h * HD:(h + 1) * HD] * c_vec[h * HD:(h + 1) * HD]).sum()
                     for h in range(NH)])              # [4]

    xp = np.zeros((NPAD, P), np.float32)
    xp[perm[:N]] = np.asarray(x, np.float32)[:N]
    u_tab = (dinv_perm[:, None] * xp).astype(bf16)     # [NPAD, 128]

    sig_cols = sigma_perm.reshape(NBLK, P).T           # [P, NBLK]
    dinv_cols = dinv_perm.reshape(NBLK, P).T

    meta = dict(NPAD=NPAD, NBLK=NBLK, NBC=NBC, HALF=HALF, TT=TT,
                TL=tuple(int(v) for v in TL), TH=tuple(int(v) for v in TH),
                col_lo=tuple(int(v) for v in col_lo),
                col_hi=tuple(int(v) for v in col_hi),
                batches=tuple((tuple(b["blocks"]), b["base"], b["nlo"], b["nhi"])
                              for b in batches),
                TBMAX=TBMAX, TJMAX=TJMAX,
                NDTL=NDTL, NDTH=NDTH, NDT=NDT, EP=EP,
                has_b1=bool(np.any(np.asarray(b1) != 0)),
                has_b2=bool(np.any(np.asarray(b2) != 0)),
                bsum=bsum, invmap=invmap)

    common = {
        "u_tab": u_tab,
        "b1colT": np.asarray(b1, np.float32).reshape(2, P).T.astype(bf16),
        "w1": np.asarray(W1, np.float32).astype(bf16),                    # [128,256]
        "w2c": np.asarray(W2, np.float32).reshape(2, P, H).astype(bf16),
        "wqc": (np.asarray(Wq, np.float32).T * scale).reshape(2, P, H).astype(bf16),
        "wkc": np.asarray(Wk, np.float32).T.reshape(2, P, H).astype(bf16),
        "uc": u2.reshape(2, P, NH).astype(bf16),
        "b1r": np.asarray(b1, np.float32).reshape(1, H).astype(bf16),
        "b2r": np.asarray(b2, np.float32).reshape(1, H).astype(bf16),
        "bqr": (np.asarray(bq, np.float32) * scale).reshape(1, H).astype(bf16),
        "bkr": np.asarray(bk, np.float32).reshape(1, H).astype(bf16),
        "betar": beta.reshape(1, NH).astype(np.float32),
        "iota_row": np.tile(np.arange(P, dtype=np.float32).astype(bf16)[None, :],
                            (P, 1)),
        "ident_bf": np.eye(P, dtype=np.float32).astype(bf16),
        "ident_f32": np.eye(P, dtype=np.float32),
    }
    in_maps = []
    for c in range(NCORES):
        m = dict(common)
        m["gidx"] = gvals_core[c]
        m["dstloc"] = dloc_core[c]
        m["spidx"] = spidx_core[c]
        m["dpidx"] = dpidx_core[c]
        m["sigma_own"] = np.ascontiguousarray(
            sig_cols[:, c * NBC:(c + 1) * NBC]).astype(bf16)
        m["dinv_own_bf"] = np.ascontiguousarray(
            dinv_cols[:, c * NBC:(c + 1) * NBC]).astype(bf16)
        m["dinv2_own"] = np.ascontiguousarray(
            dinv_cols[:, c * NBC:(c + 1) * NBC] ** 2).astype(np.float32)
        m["dinv_rows"] = np.ascontiguousarray(
            dinv_perm[c * NBC * P:(c + 1) * NBC * P].reshape(1, NBC * P))
        m["sigma_rows"] = np.ascontiguousarray(
            sigma_perm[c * NBC * P:(c + 1) * NBC * P].reshape(1, NBC * P))
        m["dinvf_own"] = np.ascontiguousarray(
            dinv_cols[:, c * NBC:(c + 1) * NBC]).astype(np.float32)
        in_maps.append(m)
    return in_maps, meta


# ----------------------------------------------------------------------------
# program builder
# ----------------------------------------------------------------------------
def build_program(meta):
    NPAD, NBLK, NBC, HALF, TT = (meta[k] for k in
                                 ("NPAD", "NBLK", "NBC", "HALF", "TT"))
    TL, TH, col_lo, col_hi = (meta[k] for k in ("TL", "TH", "col_lo", "col_hi"))
    TBMAX, TJMAX = meta["TBMAX"], meta["TJMAX"]
    NDTL, NDTH, NDT = meta["NDTL"], meta["NDTH"], meta["NDT"]
    H = HIDDEN

    nc = bacc.Bacc("TRN2", target_bir_lowering=False, debug=False,
                   num_devices=NCORES)

    def din(name, shape, dt):
        return nc.dram_tensor(name, shape, dt, kind="ExternalInput")

    u_in = din("u_tab", [NPAD, P], BF)
    w1 = din("w1", [P, H], BF)
    w2c = din("w2c", [2, P, H], BF)
    wqc = din("wqc", [2, P, H], BF)
    wkc = din("wkc", [2, P, H], BF)
    uc = din("uc", [2, P, NH], BF)
    b1r = din("b1r", [1, H], BF)
    b2r = din("b2r", [1, H], BF)
    bqr = din("bqr", [1, H], BF)
    bkr = din("bkr", [1, H], BF)
    betar = din("betar", [1, NH], F32)
    iota_in = din("iota_row", [P, P], BF)
    identb_in = din("ident_bf", [P, P], BF)
    identf_in = din("ident_f32", [P, P], F32)
    gidx_in = din("gidx", [P, TT * 8], I16)
    dstloc_in = din("dstloc", [P, TT], BF)
    spidx_in = din("spidx", [P, NDT * 8], I16)
    dpidx_in = din("dpidx", [P, NDT * 8], I16)
    dinv2_in = din("dinv2_own", [P, NBC], F32)
    dinvf_in = din("dinvf_own", [P, NBC], F32)
    dinvrows_in = din("dinv_rows", [1, NBC * P], F32)
    sigrows_in = din("sigma_rows", [1, NBC * P], F32)
    b1colT_in = din("b1colT", [P, 2], BF)

    out_t = nc.dram_tensor("out", [NDT * P], F32, kind="ExternalOutput")

    hw2d_shard = nc.dram_tensor("hw2d_shard", [NBC * P, H], BF, kind="Internal")
    hw2d_full = nc.dram_tensor("hw2d_full", [NPAD, H], BF, kind="Internal",
                               addr_space="Shared")
    tq_shard = nc.dram_tensor("tq_shard", [NBC * P, TROW], BF, kind="Internal")
    tk_shard = nc.dram_tensor("tk_shard", [NBC * P, TROW], BF, kind="Internal")
    tk_full = nc.dram_tensor("tk_full", [NPAD, TROW], BF, kind="Internal",
                             addr_space="Shared")

    AG = mybir.AluOpType
    ACTF = mybir.ActivationFunctionType
    with tile.TileContext(nc) as tc:
        with tc.tile_pool(name="sb", bufs=1) as res, \
             tc.tile_pool(name="wk", bufs=3) as wk, \
             tc.tile_pool(name="gp", bufs=2) as gp, \
             tc.tile_pool(name="dp", bufs=3) as dpool, \
             tc.tile_pool(name="ps", bufs=6, space="PSUM") as psp, \
             tc.tile_pool(name="pt", bufs=2, space="PSUM") as ptp:

            # ---------------- residents
            def load(name, src, shape, dt):
                t = res.tile(shape, dt, tag=name)
                nc.sync.dma_start(t[:], src[:])
                return t

            w1_t = load("w1", w1, [P, H], BF)

            def load2(name, src, width, dt):
                t = res.tile([P, 2 * width], dt, tag=name)
                for k in range(2):
                    nc.sync.dma_start(t[:, k * width:(k + 1) * width], src[k])
                return t

            w2_t = load2("w2c", w2c, H, BF)
            wq_t = load2("wqc", wqc, H, BF)
            wk_t = load2("wkc", wkc, H, BF)
            uc_t = load2("uc", uc, NH, BF)
            iota_t = load("iota", iota_in, [P, P], BF)
            idb_t = load("idb", identb_in, [P, P], BF)
            idf_t = load("idf", identf_in, [P, P], F32)
            gidx_t = load("gidx", gidx_in, [P, TT * 8], I16)
            dstloc_t = load("dstloc", dstloc_in, [P, TT], BF)
            spidx_t = load("spidx", spidx_in, [P, NDT * 8], I16)
            dpidx_t = load("dpidx", dpidx_in, [P, NDT * 8], I16)
            dinv2_t = load("dinv2", dinv2_in, [P, NBC], F32)
            dinvf_t = load("dinvf", dinvf_in, [P, NBC], F32)
            b1col_t = load("b1colT", b1colT_in, [P, 2], BF)

            def loadb(name, src):
                t = res.tile([P, H], BF, tag=name)
                nc.sync.dma_start(t[:], src[:].to_broadcast((P, H)))
                return t

            bq_t = loadb("bq", bqr)
            bk_t = loadb("bk", bkr)
            b2b_t = loadb("b2b", b2r) if meta["has_b2"] else None
            beta_b = res.tile([P, NH], F32, tag="betab")
            nc.sync.dma_start(beta_b[:], betar[:].to_broadcast((P, NH)))

            colbuf = res.tile([P, NDT], F32, tag="colbuf")
            bsum_t = res.tile([P, 1], F32, tag="bsum")
            nc.vector.memset(bsum_t[:], float(meta["bsum"]))

            def iota3(nt):
                return iota_t[:].rearrange("p (o w) -> p o w", o=1).to_broadcast(
                    (P, nt, P))

            def dst3(c0, nt):
                return dstloc_t[:, c0:c0 + nt].rearrange(
                    "p t -> p t ()").to_broadcast((P, nt, P))

            def build_st(j):
                stt = gp.tile([P, TJMAX * P], BF, tag="st")
                tl, th = TL[j], TH[j]
                if tl:
                    nc.vector.tensor_tensor(
                        out=stt[:, 0:tl * P].rearrange("p (t w) -> p t w", w=P),
                        in0=iota3(tl), in1=dst3(col_lo[j], tl), op=AG.is_equal)
                if th:
                    nc.vector.tensor_tensor(
                        out=stt[:, tl * P:(tl + th) * P].rearrange(
                            "p (t w) -> p t w", w=P),
                        in0=iota3(th), in1=dst3(col_hi[j], th), op=AG.is_equal)
                return stt

            # <=8 tiles (1024 idxs) per dma_gather: larger num_idxs (1536+)
            # crashes the Q7 ucode on HW (empirical; 1024 verified good).
            GMAX = 8

            def batch_gather(table_lo, table_hi, bat, width):
                blocks, base, nlo, nhi = bat
                gb = gp.tile([P, TBMAX * H], BF, tag="gbuf")
                for seg0, nseg in ((0, nlo), (nlo, nhi)):
                    tab = table_lo if seg0 == 0 else table_hi
                    for s in range(0, nseg, GMAX):
                        n = min(GMAX, nseg - s)
                        c0 = seg0 + s
                        nc.gpsimd.dma_gather(
                            gb[:, c0 * width:(c0 + n) * width].rearrange(
                                "p (t w) -> p t w", w=width),
                            tab, gidx_t[:, (base + c0) * 8:(base + c0 + n) * 8],
                            n * P, n * P, width)
                return gb

            def block_chunks(j, bat):
                """gbuf chunk indices (batch-relative) for block j, in the
                same order as build_st's columns (lows then highs)."""
                blocks, base, nlo, nhi = bat
                lo0 = col_lo[j] - base
                hi0 = nlo + (col_hi[j] - base - nlo)
                # col_hi[j] is absolute; highs start at base+nlo in the batch
                hi0 = col_hi[j] - base
                return ([lo0 + t for t in range(TL[j])] +
                        [hi0 + t for t in range(TH[j])])

            # ---------------- layer 1
            for bat in meta["batches"]:
                gb = batch_gather(u_in[0:HALF, :], u_in[HALF:NPAD, :], bat, P)
                for j in bat[0]:
                    stt = build_st(j)
                    seq = block_chunks(j, bat)
                    ps_agg = psp.tile([P, H], F32, tag="p256", space="PSUM")
                    for i, ch in enumerate(seq):
                        nc.tensor.matmul(
                            ps_agg[:, 0:P], lhsT=gb[:, ch * P:(ch + 1) * P],
                            rhs=stt[:, i * P:(i + 1) * P],
                            start=(i == 0), stop=(i == len(seq) - 1))
                    aggsb = wk.tile([P, P], BF, tag="aggsb")
                    nc.vector.tensor_copy(out=aggsb[:], in_=ps_agg[:, 0:P])
                    ph1 = psp.tile([P, H], F32, tag="p256", space="PSUM")
                    h1r = wk.tile([P, H], BF, tag="h1r")
                    if meta["has_b1"]:
                        sigbc = wk.tile([P, P], F32, tag="sigbc")
                        nc.sync.dma_start(
                            sigbc[:],
                            sigrows_in[0:1, j * P:(j + 1) * P].to_broadcast((P, P)))
                    for k in range(2):
                        nc.tensor.matmul(ph1[:, k * P:(k + 1) * P],
                                         lhsT=w1_t[:, k * P:(k + 1) * P],
                                         rhs=aggsb[:], start=True, stop=True)
                        if meta["has_b1"]:
                            # ph1 + b1[h]*sigma[d], h per-partition here
                            hpre = wk.tile([P, P], F32, tag="hpre")
                            nc.vector.tensor_scalar(
                                out=hpre[:], in0=sigbc[:],
                                scalar1=b1col_t[:, k:k + 1], op0=AG.mult)
                            nc.vector.tensor_tensor(
                                out=hpre[:], in0=hpre[:],
                                in1=ph1[:, k * P:(k + 1) * P], op=AG.add)
                            nc.scalar.activation(h1r[:, k * P:(k + 1) * P],
                                                 hpre[:], ACTF.Relu)
                        else:
                            nc.scalar.activation(h1r[:, k * P:(k + 1) * P],
                                                 ph1[:, k * P:(k + 1) * P],
                                                 ACTF.Relu)
                    pw2 = psp.tile([P, H], F32, tag="p256", space="PSUM")
                    for k in range(2):
                        nc.tensor.matmul(pw2[:], lhsT=h1r[:, k * P:(k + 1) * P],
                                         rhs=w2_t[:, k * H:(k + 1) * H],
                                         start=(k == 0), stop=(k == 1))
                    hw2b = wk.tile([P, H], BF, tag="hw2b")
                    if meta["has_b2"]:
                        # hw2 = dinv^2*pw2 + dinv*b2
                        w2pre = wk.tile([P, H], F32, tag="w2pre")
                        nc.vector.tensor_scalar(
                            out=w2pre[:], in0=b2b_t[:],
                            scalar1=dinvf_t[:, j:j + 1], op0=AG.mult)
                        w2sc = wk.tile([P, H], F32, tag="w2sc")
                        nc.scalar.activation(w2sc[:], pw2[:], ACTF.Copy,
                                             scale=dinv2_t[:, j:j + 1])
                        nc.vector.tensor_tensor(out=hw2b[:], in0=w2sc[:],
                                                in1=w2pre[:], op=AG.add)
                    else:
                        nc.scalar.activation(hw2b[:], pw2[:], ACTF.Copy,
                                             scale=dinv2_t[:, j:j + 1])
                    nc.sync.dma_start(hw2d_shard[j * P:(j + 1) * P, :], hw2b[:])

            if STAGE >= 2:
                nc.gpsimd.collective_compute(
                    "AllGather", AG.bypass, replica_groups=[list(range(NCORES))],
                    ins=[hw2d_shard[:]], outs=[hw2d_full[:]])

            # ---------------- layer 2 + decode tables
            for bat in (meta["batches"] if STAGE >= 2 else ()):
                gb = batch_gather(hw2d_full[0:HALF, :], hw2d_full[HALF:NPAD, :],
                                  bat, H)
                for j in bat[0]:
                    stt = build_st(j)
                    seq = block_chunks(j, bat)
                    psz = psp.tile([P, H], F32, tag="p256", space="PSUM")
                    for i, ch in enumerate(seq):
                        for k in range(2):
                            nc.tensor.matmul(
                                psz[:, k * P:(k + 1) * P],
                                lhsT=gb[:, ch * H + k * P:ch * H + (k + 1) * P],
                                rhs=stt[:, i * P:(i + 1) * P],
                                start=(i == 0), stop=(i == len(seq) - 1))
                    dbc = wk.tile([P, P], F32, tag="dbc")
                    nc.sync.dma_start(
                        dbc[:], dinvrows_in[0:1, j * P:(j + 1) * P].to_broadcast(
                            (P, P)))
                    zt = wk.tile([P, H], BF, tag="zt")
                    for k in range(2):
                        nc.vector.tensor_tensor(
                            out=zt[:, k * P:(k + 1) * P],
                            in0=psz[:, k * P:(k + 1) * P], in1=dbc[:], op=AG.mult)
                    psq = psp.tile([P, H], F32, tag="p256", space="PSUM")
                    psk = psp.tile([P, H], F32, tag="p256", space="PSUM")
                    pssb = psp.tile([P, H], F32, tag="p256", space="PSUM")
                    pss = pssb[:, 0:NH]
                    for k in range(2):
                        nc.tensor.matmul(psq[:], lhsT=zt[:, k * P:(k + 1) * P],
                                         rhs=wq_t[:, k * H:(k + 1) * H],
                                         start=(k == 0), stop=(k == 1))
                        nc.tensor.matmul(psk[:], lhsT=zt[:, k * P:(k + 1) * P],
                                         rhs=wk_t[:, k * H:(k + 1) * H],
                                         start=(k == 0), stop=(k == 1))
                        nc.tensor.matmul(pss, lhsT=zt[:, k * P:(k + 1) * P],
                                         rhs=uc_t[:, k * NH:(k + 1) * NH],
                                         start=(k == 0), stop=(k == 1))
                    tqt = wk.tile([P, TW], BF, tag="tqt")
                    tkt = wk.tile([P, TW], BF, tag="tkt")
                    nc.vector.tensor_tensor(out=tqt[:, 0:H], in0=psq[:],
                                            in1=bq_t[:], op=AG.add)
                    nc.vector.tensor_tensor(out=tkt[:, 0:H], in0=psk[:],
                                            in1=bk_t[:], op=AG.add)
                    qk = wk.tile([P, H], BF, tag="qk")
                    nc.vector.tensor_tensor(out=qk[:], in0=tqt[:, 0:H],
                                            in1=tkt[:, 0:H], op=AG.mult)
                    l0f = wk.tile([P, NH], F32, tag="l0f")
                    nc.vector.tensor_reduce(
                        out=l0f[:],
                        in_=qk[:].rearrange("p (h d) -> p h d", h=NH),
                        axis=mybir.AxisListType.X, op=AG.add)
                    nc.vector.tensor_copy(out=tqt[:, H:H + NH], in_=l0f[:])
                    nc.vector.tensor_tensor(out=tqt[:, H + NH:H + 2 * NH],
                                            in0=pss, in1=beta_b[:], op=AG.add)
                    nc.vector.tensor_tensor(out=tkt[:, H:H + NH],
                                            in0=pss, in1=beta_b[:], op=AG.add)
                    nc.sync.dma_start(tq_shard[j * P:(j + 1) * P, 0:TW], tqt[:])
                    nc.sync.dma_start(tk_shard[j * P:(j + 1) * P, 0:TW], tkt[:])

            if STAGE >= 3:
                nc.gpsimd.collective_compute(
                    "AllGather", AG.bypass, replica_groups=[list(range(NCORES))],
                    ins=[tk_shard[:]], outs=[tk_full[:]])

            # ---------------- decode
            NG = NDT // DG
            for g in (range(NG) if STAGE >= 3 else ()):
                low = (g * DG) < NDTL
                gq = dpool.tile([P, DG, TROW], BF, tag="gq")
                nc.gpsimd.dma_gather(
                    gq[:], tq_shard[:],
                    spidx_t[:, g * DG * 8:(g + 1) * DG * 8],
                    DG * P, DG * P, TROW)
                gk = dpool.tile([P, DG, TROW], BF, tag="gk")
                nc.gpsimd.dma_gather(
                    gk[:], tk_full[0:HALF, :] if low else tk_full[HALF:NPAD, :],
                    dpidx_t[:, g * DG * 8:(g + 1) * DG * 8],
                    DG * P, DG * P, TROW)
                prod = wk.tile([P, DG, H], BF, tag="prod")
                nc.vector.tensor_tensor(out=prod[:], in0=gq[:, :, 0:H],
                                        in1=gk[:, :, 0:H], op=AG.mult)
                l1 = wk.tile([P, DG * NH], F32, tag="l1")
                nc.vector.tensor_reduce(
                    out=l1[:], in_=prod[:].rearrange("p g (h d) -> p (g h) d", h=NH),
                    axis=mybir.AxisListType.X, op=AG.add)
                dlt = wk.tile([P, DG * NH], F32, tag="dlt")
                nc.vector.tensor_tensor(
                    out=dlt[:].rearrange("p (g h) -> p g h", h=NH),
                    in0=l1[:].rearrange("p (g h) -> p g h", h=NH),
                    in1=gq[:, :, H:H + NH], op=AG.subtract)
                a1 = wk.tile([P, DG * NH], F32, tag="a1")
                nc.scalar.activation(a1[:], dlt[:], ACTF.Sigmoid)
                ds = wk.tile([P, DG * NH], F32, tag="ds")
                nc.vector.tensor_tensor(
                    out=ds[:].rearrange("p (g h) -> p g h", h=NH),
                    in0=gk[:, :, H:H + NH], in1=gq[:, :, H + NH:H + 2 * NH],
                    op=AG.subtract)
                pr = wk.tile([P, DG * NH], F32, tag="pr")
                nc.vector.tensor_tensor(out=pr[:], in0=a1[:], in1=ds[:], op=AG.mult)
                prs = wk.tile([P, DG], F32, tag="prs")
                nc.vector.tensor_reduce(
                    out=prs[:], in_=pr[:].rearrange("p (g h) -> p g h", h=NH),
                    axis=mybir.AxisListType.X, op=AG.add)
                s0s = wk.tile([P, DG], F32, tag="s0s")
                nc.vector.tensor_reduce(
                    out=s0s[:], in_=gq[:, :, H + NH:H + 2 * NH],
                    axis=mybir.AxisListType.X, op=AG.add)
                rr = wk.tile([P, DG], F32, tag="rr")
                nc.vector.tensor_tensor(out=rr[:], in0=prs[:], in1=s0s[:],
                                        op=AG.add)
                nc.scalar.activation(colbuf[:, g * DG:(g + 1) * DG], rr[:],
                                     ACTF.Sigmoid, bias=bsum_t[:])

            if STAGE < 3:
                nc.vector.memset(colbuf[:], 0.5)
            # transpose colbuf -> out
            for c0 in range(0, NDT, P):
                w = min(P, NDT - c0)
                po = ptp.tile([P, P], F32, tag="poT", space="PSUM")
                nc.tensor.transpose(po[:w, :], colbuf[:, c0:c0 + w], idf_t[:])
                ob = wk.tile([P, P], F32, tag="ob")
                nc.vector.tensor_copy(out=ob[:w, :], in_=po[:w, :])
                nc.sync.dma_start(
                    out_t[c0 * P:(c0 + w) * P].rearrange("(a b) -> a b", b=P),
                    ob[:w, :])
    nc.compile()
    return nc


# ----------------------------------------------------------------------------
_CACHE = {}
TRACE = False
LAST_EXEC_NS = None
LAST_RES = None


def kernel(**inputs):
    import concourse.bass_utils as bass_utils
    global LAST_EXEC_NS, LAST_RES
    in_maps, meta = build_host_data(**inputs)
    key = (meta["TT"], meta["NDT"], meta["TL"], meta["TH"])
    if key not in _CACHE:
        _CACHE[key] = build_program(meta)
    nc = _CACHE[key]
    trace = bool(TRACE)
    if trace:
        try:
            import types
            try:
                import antenv.axon_hooks  # noqa: F401
            except ImportError:
                import antenv
                mod = types.ModuleType("antenv.axon_hooks")
                _h = [None]
                mod.get_axon_ntff_profile_hook = lambda: _h[0]
                mod.set_axon_ntff_profile_hook = (
                    lambda hook: _h.__setitem__(0, hook))
                sys.modules["antenv.axon_hooks"] = mod
                antenv.axon_hooks = mod
            from trn_agent_boot.trn_boot import _ntff_profile_via_ctypes
            import antenv.axon_hooks as ah
            if ah.get_axon_ntff_profile_hook() is None:
                hook = _ntff_profile_via_ctypes("/opt/axon/libaxon_pjrt.so")
                if hook is not None:
                    ah.set_axon_ntff_profile_hook(hook)
                else:
                    trace = False
        except Exception:
            trace = False
    ncores_run = int(os.environ.get("V2_CORES", str(NCORES)))
    res = bass_utils.run_bass_kernel_spmd(nc, in_maps[:ncores_run],
                                          core_ids=list(range(ncores_run)),
                                          trace=trace)
    LAST_EXEC_NS = res.exec_time_ns
    LAST_RES = res
    EP = meta["EP"]
    out = np.zeros(EP, np.float32)
    for c in range(ncores_run):
        inv = meta["invmap"][c]
        m = inv >= 0
        out[inv[m]] = res.results[c]["out"][m]
    return out
